# revision 80
# baseline (speedup 1.0000x reference)
"""Trainium2 Bass kernel for windowed multi-agent attention (Swin-style).

Full-input contract: kernel(**inputs) takes the unsharded inputs and returns
the unsharded output. Internally shards over the H axis across 8 NeuronCores
(fully data-parallel over window rows), builds one SPMD Bass program, and
runs it via run_bass_kernel_spmd.

Host-side, x is pre-reordered into per-core token-major layout (bf16):
  xs[core][c2, p128, strip4, (w32 a4 i4 j4)]
so each (c, strip) loads with a single fully-contiguous DMA and the SBUF
tile is already in window-token order. The output uses the same layout in
reverse (bf16), reassembled to NCHW f32 on the host.

Per window (4x4 spatial, 4 agents => T=64 tokens; the padded 5th agent is
masked out everywhere in the reference, so it is dropped):
  xw (64, 256) -> qkv -> 4 heads of d=64 -> softmax(q k^T * scale + bias) v
  -> out proj (256, 256).

The attention core is computed fully in "transposed" (S^T) space to avoid
PE transposes:
  ST = k^T q                      (t_k on partitions, t_q on cols)
  E  = exp(ST) * exp(B)^T         (bias multiplicative, host-precomputed)
  Z  = sel^T @ E                  (PE matmul with 64x64 block mask ->
                                   per-(window, t_q) sums replicated across
                                   the 64 t_k partitions of each half)
  P^T = E * reciprocal(Z)
  o^T = v^T P^T                   (lhsT = v with t_k on partitions)
  out^T = Wo^T o^T
"""

import numpy as np

HEADS = 4
WIN = 4
MAX_N = 5
DIM = 256
N_AGENTS = 4
H = W = 128
N_CORES = 8
T = N_AGENTS * WIN * WIN          # 64 valid tokens per window
HS = 16                           # H rows per core
N_STRIPS = 4                      # window-rows per core (4 H-rows each)
N_GROUPS = 4                      # groups of 8 windows per strip
GW = 8                            # windows per group
NT = GW * T                       # tokens per group = 512
SW = 32                           # windows per strip
STOK = SW * T                     # tokens per strip = 2048


def _rel_pos_index(N, wh, ww, md, mh, mw):
    cd, ch, cw = np.arange(N), np.arange(wh), np.arange(ww)
    coords = np.stack(np.meshgrid(cd, ch, cw, indexing="ij")).reshape(3, -1)
    rel = (coords[:, :, None] - coords[:, None, :]).transpose(1, 2, 0).astype(np.int64)
    rel[..., 0] += md - 1
    rel[..., 1] += mh - 1
    rel[..., 2] += mw - 1
    rel[..., 0] *= (2 * mh - 1) * (2 * mw - 1)
    rel[..., 1] *= 2 * mw - 1
    return rel.sum(-1)


def _build_biasT_stacks(bias_table):
    """Transposed multiplicative bias stacks, one per head-pair 'stack'.

    Returns (2, 128, NT) f32 where
      out[st, hh*64 + tk, w*64 + tq] = exp(B_{2*st+hh}[tq, tk])
    (independent of w: identical 64x64 blocks tiled along the 8 windows)."""
    rpi = _rel_pos_index(MAX_N, WIN, WIN, MAX_N, WIN, WIN)   # (80, 80)
    b = bias_table[rpi]                                      # (80, 80, HEADS)
    b = b[:T, :T].transpose(2, 0, 1).astype(np.float32)      # (HEADS, tq, tk)
    out = np.zeros((2, 128, NT), np.float32)
    for st in range(2):
        for hh in range(2):
            blk = np.exp(b[2 * st + hh].T)                   # (tk, tq)
            for w in range(GW):
                out[st, hh * 64:(hh + 1) * 64, w * T:(w + 1) * T] = blk
    return np.ascontiguousarray(out)


def _patch_tile_drain():
    """Walrus in this container rejects >1 sync-wait on the TileContext tail
    drain; split the waits across individual SP nops instead."""
    from concourse import tile as tile_mod
    from concourse.vector_clock import ScopedClock, VectorClock
    if getattr(tile_mod.TileContext, "_drain_patched", False):
        return

    def _patched(self, tick_clock, wait_clock):
        gc_ = tick_clock.global_clock
        n = len(gc_)
        for proc in range(n):
            tick = gc_[proc]
            if tick <= 0:
                continue
            vc = VectorClock([0] * n)
            vc.require_at_least(proc, tick)
            nop_inst = self.nc.sync.nop(nofuse=True)
            wait_clock.add_sem_waits(nop_inst.ins, ScopedClock({None: vc}))
        self.nc.sync.drain()
        self.nc.all_engine_barrier()
        popped = self.nc._tile_sem_poison_stack.pop()
        assert popped is self._sem_poison
        self.nc.clear_and_free_semaphores(list(self.sems.allocated().values()))
        self.nc.all_engine_barrier()

    tile_mod.TileContext._drain_and_barrier = _patched
    tile_mod.TileContext._drain_patched = True


def _split_multi_waits(nc):
    """Walrus here allows only one sync-wait per instruction. Rewrite the BIR
    json: for each instruction with >1 on_wait, hoist the extras onto fresh
    single-wait Nops inserted just before it on the same engine."""
    import orjson
    orig = nc.to_json_bytes

    def patched():
        bj = orjson.loads(orig())
        counter = [0]
        for fn in bj.get("functions", []):
            for blk in fn.get("blocks", []):
                insts = blk.get("instructions", [])
                out = []
                for inst in insts:
                    si = inst.get("sync_info") or {}
                    waits = si.get("on_wait") or []
                    if len(waits) > 1:
                        for w in waits[:-1]:
                            counter[0] += 1
                            out.append({
                                "name": f"WSPL-{counter[0]}",
                                "opcode": "NoOp",
                                "engine": inst["engine"],
                                "ins": [],
                                "outs": [],
                                "sync_info": {"on_update": [], "on_wait": [w]},
                            })
                        si["on_wait"] = [waits[-1]]
                    out.append(inst)
                blk["instructions"] = out
        return orjson.dumps(bj)

    nc.to_json_bytes = patched
    return nc


def build_nc():
    import os
    from concourse import bass, mybir
    from concourse.tile import TileContext
    _patch_tile_drain()
    KMULS = os.environ.get("KMULS", "dve")      # dve | pool | split
    KCHUNK0 = os.environ.get("KCHUNK0", "1") == "1"
    KDIV = os.environ.get("KDIV", "0") == "1"   # tensor_tensor divide for norm
    KOS = os.environ.get("KOS", "split")        # act | split : U->OS evac engines
    KVSB = os.environ.get("KVSB", "split")      # dve | act | split : VP evac
    KOUTCHUNK = os.environ.get("KOUTCHUNK", "1") == "1"
    KFP8 = os.environ.get("KFP8", "0") == "1"   # DoubleRow fp8 V-proj + out-proj
    KQK = os.environ.get("KQK", "act")          # act | split : q/k evac engines
    KO8 = os.environ.get("KO8", "dve")          # dve | split : o8 evac engines
    KPOSTN = os.environ.get("KPOSTN", "0") == "1"  # normalize after AV
    KPRIO = int(os.environ.get("KPRIO", "0"))      # high-priority softmax chain
    KWARM = int(os.environ.get("KWARM", "5"))      # PE warm-up dummy matmuls
    KPOSTLAST = int(os.environ.get("KPOSTLAST", "1"))  # post-AV norm, final N groups
    KVSWQ = os.environ.get("KVSWQ", "sp")          # sp | pool : vSW swap DMA queue
    _kt = os.environ.get("KTAIL", "2")
    KTAIL = _kt != "0"                              # final-group oB on Act
    KTAILOS = _kt == "1"                            # + half-group OS/DMA drain

    F32 = mybir.dt.float32
    BF16 = mybir.dt.bfloat16
    EXP = mybir.ActivationFunctionType.Exp

    nc = bass.Bass("TRN2", target_bir_lowering=False, debug=False,
                   num_devices=N_CORES)

    F8 = mybir.dt.float8e4
    # packed bf16 constant blobs (17 small HWDGE-serialized DMAs made the
    # pipeline fill weights-bound). Split hot-first: wc1 = [wq wk] needed by
    # the first projections; wc2 = [wv wo biasT sel] can land later.
    wc1_d = nc.dram_tensor("wcat1", [128, 1024], BF16, kind="ExternalInput").ap()
    wc2_d = nc.dram_tensor("wcat2", [128, 2176], BF16, kind="ExternalInput").ap()
    if KFP8:
        # fp8-only input stream (halves input DMA bytes); Q/K take the fp8
        # tokens as the moving operand against bf16 weights, V/out-proj use
        # DoubleRow with [wv8 (2x256) | wo8 (2x256)] packed weights
        xs8_d = nc.dram_tensor("xs8", [2, 128, N_STRIPS, STOK], F8, kind="ExternalInput").ap()
        w8_d = nc.dram_tensor("w8", [128, 1024], F8, kind="ExternalInput").ap()
    else:
        xs_d = nc.dram_tensor("xs", [2, 128, N_STRIPS, STOK], BF16, kind="ExternalInput").ap()
    out_d = nc.dram_tensor("out", [2, 128, N_STRIPS, STOK], BF16, kind="ExternalOutput").ap()

    from contextlib import ExitStack
    with TileContext(nc) as tc, ExitStack() as _stk:
        cpool = _stk.enter_context(tc.tile_pool(name="consts", bufs=1))
        wcat1 = cpool.tile([128, 1024], BF16, name="wcat1", tag="wcat1")
        wcat2 = cpool.tile([128, 2176], BF16, name="wcat2", tag="wcat2")
        nc.sync.dma_start(out=wcat1[:], in_=wc1_d)
        if KFP8:
            w8 = cpool.tile([128, 1024], F8, name="w8", tag="w8")
            nc.sync.dma_start(out=w8[:], in_=w8_d)
            # 3D [p, 2(c-half), N] views for DoubleRow
            Wv8 = w8[:, 0:512].rearrange("p (two n) -> p two n", two=2)
            Wo8 = [w8[:, 512 + h * 256: 512 + (h + 1) * 256].rearrange(
                "p (two n) -> p two n", two=2) for h in range(2)]

        if KWARM > 0:
            # scratch operands for PE warm-up matmuls: the cost model ramps
            # the PE clock only after ~3us of continuous activity, so spin
            # the array on zeros during the DMA fill (result is overwritten
            # by group 0's first start=True accumulation)
            warm_sb = cpool.tile([128, 640], BF16, name="warm", tag="warm")
            nc.vector.memzero(warm_sb[:])
        Wq = [[wcat1[:, (c * 2 + h) * 128:(c * 2 + h) * 128 + 128] for h in range(2)] for c in range(2)]
        Wk = [[wcat1[:, 512 + (c * 2 + h) * 128: 512 + (c * 2 + h) * 128 + 128] for h in range(2)] for c in range(2)]

        def _wslice(base, n):
            return wcat2[:, base:base + n]
        Wv = [_wslice(c * 256, 256) for c in range(2)]
        Wo = [[_wslice(512 + (c * 2 + h) * 128, 128) for h in range(2)] for c in range(2)]
        biasT = [_wslice(1024 + s * NT, NT) for s in range(2)]
        sel = _wslice(2048, 128)

        strip = _stk.enter_context(tc.tile_pool(name="strip", bufs=2))
        grp = _stk.enter_context(tc.tile_pool(name="grp", bufs=3))
        # Stage-split PSUM pools (8 banks total): early tiles (Q/K/V/S, 8
        # allocs per group) rotate 5 banks; late tiles (Z/OT/U, 6 allocs)
        # rotate 3. FIFO pairings then couple early-with-early across
        # groups (next group's Q-proj no longer waits on this group's
        # out-projection evacuations to free a bank).
        KPSE = int(os.environ.get("KPSE", "5"))
        pse = _stk.enter_context(tc.tile_pool(name="pse", bufs=KPSE, space="PSUM"))
        psl = _stk.enter_context(tc.tile_pool(name="psl", bufs=8 - KPSE, space="PSUM"))

        def load_strip(s, chunked=False, after_first=None):
            if KFP8:
                Traw8 = strip.tile([128, 2, STOK], F8, name="traw8", tag="traw8")
                Traw = [Traw8[:, c, :] for c in range(2)]
                if chunked:
                    for g in range(N_GROUPS):
                        gsl = slice(g * NT, (g + 1) * NT)
                        for c in range(2):
                            nc.sync.dma_start(out=Traw8[:, c, gsl], in_=xs8_d[c, :, s, gsl])
                        if g == 0 and after_first is not None:
                            after_first()
                else:
                    for c in range(2):
                        nc.sync.dma_start(out=Traw8[:, c, :], in_=xs8_d[c, :, s, :])
                return Traw, Traw8
            Traw = [strip.tile([128, STOK], BF16, name=f"traw{c}", tag=f"traw{c}") for c in range(2)]
            if chunked:
                # per-group chunks so group 0 compute starts early
                for g in range(N_GROUPS):
                    gsl = slice(g * NT, (g + 1) * NT)
                    for c in range(2):
                        nc.sync.dma_start(out=Traw[c][:, gsl], in_=xs_d[c, :, s, gsl])
                    if g == 0 and after_first is not None:
                        after_first()
            else:
                for c in range(2):
                    nc.sync.dma_start(out=Traw[c][:], in_=xs_d[c, :, s, :])
            return Traw, None

        # bulk constants (wv/wo/bias/sel) land right after strip 0's first
        # group of chunks: [wc1, g0 chunks, wc2, g1-3 chunks, ...]
        pending = load_strip(0, chunked=KCHUNK0,
                             after_first=lambda: nc.sync.dma_start(out=wcat2[:], in_=wc2_d))
        if not KCHUNK0:
            nc.sync.dma_start(out=wcat2[:], in_=wc2_d)
        for s in range(N_STRIPS):
            Traw, Traw8 = pending
            if s + 1 < N_STRIPS:
                pending = load_strip(s + 1)
            OS = [strip.tile([128, STOK], BF16, name=f"os{c}", tag=f"os{c}") for c in range(2)]

            for g in range(N_GROUPS):
                gt = slice(g * NT, (g + 1) * NT)
                tok = [Traw[c][:, gt] for c in range(2)]
                # ---- q/k projections (head-pair stacks on partitions) ----
                # PSUM tile alloc order must follow stage order (see pool note)
                QA = pse.tile([128, NT], F32, name="QA", tag="ps")
                QB = pse.tile([128, NT], F32, name="QB", tag="ps")
                KA = pse.tile([128, NT], F32, name="KA", tag="ps")
                KB = pse.tile([128, NT], F32, name="KB", tag="ps")
                VP = [pse.tile([128, 512], F32, name=f"VP{i}", tag="ps") for i in range(2)]
                if KWARM > 0 and s == 0 and g == 0:
                    for _ in range(KWARM):
                        nc.tensor.matmul(QA[:], warm_sb[:, 512:640], warm_sb[:, 0:512],
                                         start=True, stop=True)
                for dst, Wsrc, h in ((QA, Wq, 0), (KA, Wk, 0), (QB, Wq, 1), (KB, Wk, 1)):
                    for c in range(2):
                        nc.tensor.matmul(dst[:], Wsrc[c][h], tok[c], start=(c == 0), stop=(c == 1))
                # ---- v (token-rows form), windows pair-stacked on partitions ----
                for p in range(4):
                    dst = VP[p // 2][:, (p % 2) * 256:(p % 2 + 1) * 256]
                    if KFP8:
                        lhsT8 = Traw8[:, :, g * NT + p * 128: g * NT + (p + 1) * 128]
                        nc.tensor.matmul(dst, lhsT8, Wv8, start=True, stop=True,
                                         perf_mode=mybir.MatmulPerfMode.DoubleRow)
                    else:
                        for c in range(2):
                            lhsT = Traw[c][:, g * NT + p * 128: g * NT + (p + 1) * 128]
                            nc.tensor.matmul(dst, lhsT, Wv[c], start=(c == 0), stop=(c == 1))

                qA = grp.tile([128, NT], BF16, name="qA", tag="qA")
                qB = grp.tile([128, NT], BF16, name="qB", tag="qB")
                kA = grp.tile([128, NT], BF16, name="kA", tag="kA")
                kB = grp.tile([128, NT], BF16, name="kB", tag="kB")
                nc.scalar.copy(qA[:], QA[:])
                nc.scalar.copy(kA[:], KA[:])
                if KQK == "split":
                    nc.vector.tensor_copy(qB[:], QB[:])
                    nc.vector.tensor_copy(kB[:], KB[:])
                else:
                    nc.scalar.copy(qB[:], QB[:])
                    nc.scalar.copy(kB[:], KB[:])
                # v in SBUF: window-pair tokens on partitions x 2x256 chans;
                # vSW = half-swapped copy (window tokens at the opposite
                # partition half) via SBUF->SBUF DMA so every AV matmul can be
                # partition-diagonal (off-diagonal PE tile placement faults).
                vSB1 = grp.tile([128, 1024], BF16, name="vSB1", tag="vSB1")
                if KVSB == "split":
                    nc.scalar.copy(vSB1[:, 0:512], VP[0][:])
                    nc.vector.tensor_copy(vSB1[:, 512:1024], VP[1][:])
                elif KVSB == "act":
                    nc.scalar.copy(vSB1[:, 0:512], VP[0][:])
                    nc.scalar.copy(vSB1[:, 512:1024], VP[1][:])
                else:
                    nc.vector.tensor_copy(vSB1[:, 0:512], VP[0][:])
                    nc.vector.tensor_copy(vSB1[:, 512:1024], VP[1][:])
                vSW = grp.tile([128, 1024], BF16, name="vSW", tag="vSW")
                _vswq = nc.gpsimd if KVSWQ == "pool" else nc.sync
                _vswq.dma_start(out=vSW[0:64, :], in_=vSB1[64:128, :])
                _vswq.dma_start(out=vSW[64:128, :], in_=vSB1[0:64, :])

                # ---- sim: ST = k^T q, (hh, t_k) partitions x (w, t_q) cols
                # (partition-diagonal: out/lhsT/rhs all at base hh*64)
                SA = pse.tile([128, NT], F32, name="SA", tag="ps")
                SB = pse.tile([128, NT], F32, name="SB", tag="ps")
                for w in range(GW):
                    wt = slice(w * T, (w + 1) * T)
                    for hh in range(2):
                        hsl = slice(hh * 64, (hh + 1) * 64)
                        nc.tensor.matmul(SA[hsl, wt], kA[hsl, wt], qA[hsl, wt], start=True, stop=True)
                        nc.tensor.matmul(SB[hsl, wt], kB[hsl, wt], qB[hsl, wt], start=True, stop=True)

                # ---- softmax pieces in transposed space ----
                from contextlib import nullcontext
                _hp = (tc.high_priority(offset=KPRIO) if KPRIO > 0 else nullcontext())
                EuA = grp.tile([128, NT], BF16, name="EuA", tag="EuA")
                EuB = grp.tile([128, NT], BF16, name="EuB", tag="EuB")
                EBA = grp.tile([128, NT], BF16, name="EBA", tag="EBA")
                EBB = grp.tile([128, NT], BF16, name="EBB", tag="EBB")
                with _hp:
                    nc.scalar.activation(EuA[:], SA[:], EXP)
                    nc.scalar.activation(EuB[:], SB[:], EXP)
                    _bias_eng = nc.gpsimd if KMULS in ("pool", "split") else nc.vector
                    _bias_eng.tensor_mul(EBA[:], EuA[:], biasT[0])
                    _bias_eng.tensor_mul(EBB[:], EuB[:], biasT[1])

                ZA = psl.tile([128, NT], F32, name="ZA", tag="ps")
                ZB = psl.tile([128, NT], F32, name="ZB", tag="ps")
                nc.tensor.matmul(ZA[:], sel, EBA[:], start=True, stop=True)
                nc.tensor.matmul(ZB[:], sel, EBB[:], start=True, stop=True)
                rzA = grp.tile([128, NT], BF16, name="rzA", tag="rzA")
                rzB = grp.tile([128, NT], BF16, name="rzB", tag="rzB")
                with nc.allow_low_precision(reason="softmax denom, bf16 ok at 2e-2 tol"):
                    nc.vector.reciprocal(rzA[:], ZA[:])
                    nc.vector.reciprocal(rzB[:], ZB[:])
                postn = KPOSTN or (s == N_STRIPS - 1 and g >= N_GROUPS - KPOSTLAST)
                if postn:
                    # normalize after AV: rz is replicated across each
                    # partition half, and o^T partitions (hh, dv) live in
                    # exactly that half -> fold 1/Z into the o evacuation.
                    # (used for the drain-critical final group: its AV no
                    # longer waits on recip+norm)
                    NTA, NTB = EBA, EBB
                else:
                    NTA = grp.tile([128, NT], BF16, name="NTA", tag="NTA")
                    NTB = grp.tile([128, NT], BF16, name="NTB", tag="NTB")
                    _norm_eng = nc.gpsimd if KMULS == "pool" else nc.vector
                    _norm_eng.tensor_mul(NTA[:], EBA[:], rzA[:])
                    _norm_eng.tensor_mul(NTB[:], EBB[:], rzB[:])

                # ---- o^T = v^T P^T: (hh, dv) partitions x (w, t_q) cols ----
                # diagonal at base hh*64; pick vSB1 or the half-swapped vSW so
                # window w's tokens sit at partition half hh.
                OTA = psl.tile([128, NT], F32, name="OTA", tag="ps")
                OTB = psl.tile([128, NT], F32, name="OTB", tag="ps")
                for w in range(GW):
                    wt = slice(w * T, (w + 1) * T)
                    p = w // 2
                    cbase = (p // 2) * 512 + (p % 2) * 256
                    for hh in range(2):
                        hsl = slice(hh * 64, (hh + 1) * 64)
                        vt = vSB1 if (w % 2) == hh else vSW
                        nc.tensor.matmul(
                            OTA[hsl, wt],
                            vt[hsl, cbase + hh * 64: cbase + (hh + 1) * 64],
                            NTA[hsl, wt], start=True, stop=True)
                        nc.tensor.matmul(
                            OTB[hsl, wt],
                            vt[hsl, cbase + 128 + hh * 64: cbase + 128 + (hh + 1) * 64],
                            NTB[hsl, wt], start=True, stop=True)
                # ---- out projection: out^T (cout, tokens) ----
                UA = psl.tile([128, NT], F32, name="UA", tag="ps")
                UB = psl.tile([128, NT], F32, name="UB", tag="ps")
                def _oevac(dst, src, rz, eng):
                    if postn:
                        eng.tensor_mul(dst, src, rz)
                    elif eng is nc.vector:
                        nc.vector.tensor_copy(dst, src)
                    else:
                        nc.scalar.copy(dst, src)

                if KFP8:
                    o8 = grp.tile([128, 2, NT], F8, name="o8", tag="o8")
                    _oevac(o8[:, 0, :], OTA[:], rzA[:], nc.vector)
                    _oevac(o8[:, 1, :], OTB[:], rzB[:],
                           nc.vector if (KPOSTN or KO8 != "split") else nc.scalar)
                    nc.tensor.matmul(UA[:], Wo8[0], o8[:], start=True, stop=True,
                                     perf_mode=mybir.MatmulPerfMode.DoubleRow)
                    nc.tensor.matmul(UB[:], Wo8[1], o8[:], start=True, stop=True,
                                     perf_mode=mybir.MatmulPerfMode.DoubleRow)
                else:
                    last = KTAIL and s == N_STRIPS - 1 and g == N_GROUPS - 1
                    oA = grp.tile([128, NT], BF16, name="oA", tag="oA")
                    oB = grp.tile([128, NT], BF16, name="oB", tag="oB")
                    _oevac(oA[:], OTA[:], rzA[:], nc.vector)
                    # final group: oB on Act so the two evacs run in parallel
                    # (nothing else left for Act in the pipeline drain)
                    _oevac(oB[:], OTB[:], rzB[:],
                           nc.vector if postn else (nc.scalar if last else nc.vector))
                    for c, o_ in ((0, oA), (1, oB)):
                        st_, sp_ = (c == 0), (c == 1)
                        nc.tensor.matmul(UA[:], Wo[c][0], o_[:], start=st_, stop=sp_)
                        nc.tensor.matmul(UB[:], Wo[c][1], o_[:], start=st_, stop=sp_)
                if not KFP8 and KTAILOS and s == N_STRIPS - 1 and g == N_GROUPS - 1:
                    # drain the final group in half-group slices so the last
                    # output DMAs start while the second half still evacuates
                    for h in range(2):
                        hsl = slice(g * NT + h * 256, g * NT + (h + 1) * 256)
                        usl = slice(h * 256, (h + 1) * 256)
                        nc.scalar.copy(OS[0][:, hsl], UA[:, usl])
                        nc.vector.tensor_copy(OS[1][:, hsl], UB[:, usl])
                        for c in range(2):
                            nc.sync.dma_start(out=out_d[c, :, s, hsl], in_=OS[c][:, hsl])
                    continue
                nc.scalar.copy(OS[0][:, gt], UA[:])
                if KOS == "split":
                    nc.vector.tensor_copy(OS[1][:, gt], UB[:])
                else:
                    nc.scalar.copy(OS[1][:, gt], UB[:])
                if KOUTCHUNK:
                    for c in range(2):
                        nc.sync.dma_start(out=out_d[c, :, s, gt], in_=OS[c][:, gt])

            if not KOUTCHUNK:
                for c in range(2):
                    nc.sync.dma_start(out=out_d[c, :, s, :], in_=OS[c][:])

    return _split_multi_waits(nc)


_NC_CACHE = None


def kernel(x, w_qkv, w_out, bias_table, _want_trace=False):
    global _NC_CACHE
    import os
    import ml_dtypes
    from concourse.bass_utils import run_bass_kernel_spmd

    BF = ml_dtypes.bfloat16
    x = np.asarray(x, dtype=np.float32)
    w_qkv = np.asarray(w_qkv, dtype=np.float32)
    w_out = np.asarray(w_out, dtype=np.float32)
    bias_table = np.asarray(bias_table, dtype=np.float32)

    scale = (DIM // HEADS) ** -0.5
    wq = w_qkv[:, 0:DIM] * scale
    wk = w_qkv[:, DIM:2 * DIM]
    wv = w_qkv[:, 2 * DIM:3 * DIM]
    biasT = _build_biasT_stacks(bias_table)
    selm = np.zeros((128, 128), np.float32)
    selm[:64, :64] = 1.0
    selm[64:, 64:] = 1.0

    # packed constant blobs: wcat1 = [wq wk], wcat2 = [wv wo biasT sel];
    # each 256-row weight split into two 128-partition column blocks
    def _split_c(wmat):       # (256, n) -> (128, 2*n)
        return np.concatenate([wmat[0:128, :], wmat[128:256, :]], axis=1)
    wcat1 = np.concatenate([_split_c(wq), _split_c(wk)], axis=1).astype(BF)
    wcat2 = np.concatenate([
        _split_c(wv), _split_c(w_out), biasT[0], biasT[1], selm,
    ], axis=1).astype(BF)
    assert wcat1.shape == (128, 1024) and wcat2.shape == (128, 2176)

    # host reorder: x (a, C, H, W) -> per-core (c2, p128, s4, (w a i j)) bf16
    xr32 = x.reshape(N_AGENTS, 2, 128, N_CORES, N_STRIPS, WIN, SW, WIN)
    xr32 = xr32.transpose(3, 1, 2, 4, 6, 0, 5, 7)     # (m, c, p, s, w, a, i, j)
    xr32 = np.ascontiguousarray(xr32.reshape(N_CORES, 2, 128, N_STRIPS, STOK))
    kfp8 = os.environ.get("KFP8", "0") == "1"
    xr = None if kfp8 else xr32.astype(BF)
    if kfp8:
        F8NP = ml_dtypes.float8_e4m3
        xr8 = xr32.astype(F8NP)
        woA = np.concatenate([w_out[0:128, 0:128], w_out[128:256, 0:128]], axis=1)
        woB = np.concatenate([w_out[0:128, 128:256], w_out[128:256, 128:256]], axis=1)
        w8 = np.concatenate([_split_c(wv), woA, woB], axis=1).astype(F8NP)
        assert w8.shape == (128, 1024)

    if _NC_CACHE is None:
        _NC_CACHE = build_nc()
    nc = _NC_CACHE

    in_maps = []
    for m in range(N_CORES):
        im = {"wcat1": wcat1, "wcat2": wcat2}
        if kfp8:
            im["xs8"] = xr8[m]
            im["w8"] = w8
        else:
            im["xs"] = xr[m]
        in_maps.append(im)
    res = run_bass_kernel_spmd(nc, in_maps, list(range(N_CORES)), trace=_want_trace)
    out = np.empty((N_AGENTS, DIM, H, W), dtype=np.float32)
    for m in range(N_CORES):
        o = np.asarray(res.results[m]["out"], dtype=np.float32)
        o = o.reshape(2, 128, N_STRIPS, SW, N_AGENTS, WIN, WIN)   # c p s w a i j
        o = o.transpose(4, 0, 1, 2, 5, 3, 6)                      # a c p s i w j
        out[:, :, m * HS:(m + 1) * HS, :] = o.reshape(N_AGENTS, DIM, HS, W)
    if _want_trace:
        return out, res
    return out
